# revision 21
# baseline (speedup 1.0000x reference)
"""Trainium2 Bass kernel for a Qwen2-VL vision transformer block.

Strategy: 8-way sequence-parallel across NeuronCores. Each core owns a
256-row shard of the 2048-token sequence and the full weights. K/V for
the full sequence are exchanged with a single fp8 AllGather; every other
stage is perfectly partitioned.

Precision plan (rel-err budget 2e-2, measured ~2e-3):
  - Residual stream and LayerNorm statistics in fp32.
  - MLP matmuls in bf16 (dominant error term, ~1.7e-3).
  - QKV and O projections in fp8e4m3 with DoubleRow perf mode (2x PE
    throughput, half weight DMA). Weights are pre-scaled by 64 on the
    host so they sit in e4m3's normal range; the 1/64 is folded into the
    PSUM->SBUF copies.
  - Attention (Q/K/V/exp(scores)) in fp8e4m3: halves the AllGather and
    the K/V reload traffic. Error contribution is ~3e-4 at the output
    because attn_out is small relative to the residual.
  - The softmax denominator is accumulated via an extra (1/32)-valued
    column appended to each head's V in the gathered layout; attn is
    scaled by 32 into bf16 (normal range for the later fp8 cast) and the
    32*64 is folded into the O-projection PSUM evacuation.

Layout notes:
  - All projections use a packed stationary activation tile
    [128, HC, seq] so DoubleRow can slice two 128-row contraction tiles
    per instruction; weights load as single [128, HC, H] DMAs.
  - Attention computes scores^T [key, query] per head (exp on the
    scalar engine straight out of PSUM), then attnV with exp(scores)
    stationary, producing attention in natural [query, head*HD] layout.
  - W1 is fully prefetched into SBUF during attention (DMA is otherwise
    idle there); W2 streams during the MLP with grouped DMAs.
"""

import sys

import numpy as np

for _p in ("/opt/trn_rl_repo",):
    if _p not in sys.path:
        sys.path.insert(0, _p)

import ml_dtypes  # noqa: E402


BF = ml_dtypes.bfloat16
F8 = ml_dtypes.float8_e4m3

B, S, H = 1, 2048, 1280
NH, HD = 16, 80
MLP = 5120
EPS = 1e-6
NCORES = 8
SL = S // NCORES            # 256 sequence rows per core
SB = SL // 128              # 2 partition blocks per core
HC = H // 128               # 10 contraction chunks over H
DR = HC // 2                # 5 DoubleRow steps over H
MC = MLP // 128             # 40 blocks over the MLP dim
KB = S // 128               # 16 key blocks over the full sequence
NCOLS = ((0, 512), (512, 512), (1024, 256))
VCOLS = ((0, 480), (480, 480), (960, 320))   # head-aligned chunks for V
SCALE = 1.0 / float(np.sqrt(np.float32(HD)))
WS = 64.0                   # host-side fp8 weight scale for QKV/O
AS = 32.0                   # on-chip attention scale (normalize trick)
HD1 = HD + 1                # V head stride incl. denominator column
KT_ELEMS = NH * HD * SL     # 327680, gathered K^T region (fp8 bytes)
V_ELEMS = SL * NH * HD1     # 331776, gathered V(+ones) region
KVE = KT_ELEMS + V_ELEMS
W1G = 8                     # W1 prefetch groups (5 mb-tiles each)
W2G = 10                    # W2 stream groups (4 mb-tiles each)


def _build_bass(use_bias):
    import bass_rust
    import concourse.bacc as bacc
    import concourse.tile as tile
    from concourse import mybir
    from concourse.masks import make_identity

    F32 = mybir.dt.float32
    BF16 = mybir.dt.bfloat16
    FP8 = mybir.dt.float8e4
    AF = mybir.ActivationFunctionType
    OP = mybir.AluOpType
    DRPM = mybir.MatmulPerfMode.DoubleRow

    nc = bacc.Bacc("TRN2", target_bir_lowering=False, debug=False,
                   num_devices=NCORES)

    x_io = nc.dram_tensor("x_loc", [SL, H], F32, kind="ExternalInput")
    cos_io = nc.dram_tensor("cosr", [SL, H], BF16, kind="ExternalInput")
    sin_io = nc.dram_tensor("sins", [SL, H], BF16, kind="ExternalInput")
    wq_io = nc.dram_tensor("wq8", [H, H], FP8, kind="ExternalInput")
    wk_io = nc.dram_tensor("wk8", [H, H], FP8, kind="ExternalInput")
    wv_io = nc.dram_tensor("wv8", [H, H], FP8, kind="ExternalInput")
    wo_io = nc.dram_tensor("wo8", [H, H], FP8, kind="ExternalInput")
    w1b_io = nc.dram_tensor("w1b", [MC, 128, HC, 128], BF16,
                            kind="ExternalInput")
    w2t_io = nc.dram_tensor("w2t", [MLP, H], BF16, kind="ExternalInput")
    bias5_io = nc.dram_tensor("bias5", [5, H], BF16, kind="ExternalInput")
    b1s_io = nc.dram_tensor("b1s", [128, MC], F32, kind="ExternalInput")
    out_io = nc.dram_tensor("out_loc", [SL, H], F32, kind="ExternalOutput")

    cc_in = nc.dram_tensor("cc_in", [KVE], FP8)
    cc_out = nc.dram_tensor("cc_out", [NCORES, KVE], FP8,
                            addr_space="Shared")

    with tile.TileContext(nc) as tc:
        const = tc.alloc_tile_pool(name="const", bufs=1)
        persist = tc.alloc_tile_pool(name="persist", bufs=1)
        misc = tc.alloc_tile_pool(name="misc", bufs=2)

        ident = const.tile([128, 128], BF16, name="ident", tag="ident")
        make_identity(nc, ident)
        ones_b = const.tile([1, 128], BF16, name="ones_b", tag="ones_b")
        nc.vector.memset(ones_b, 1.0)
        eps_t = const.tile([128, 1], F32, name="eps_t", tag="eps_t")
        nc.vector.memset(eps_t, EPS)
        bias_t = []
        if use_bias:
            for bi in range(5):
                bt = const.tile([1, H], BF16, name=f"bias{bi}",
                                tag=f"bias{bi}")
                nc.sync.dma_start(out=bt, in_=bias5_io[bi:bi + 1, :])
                bias_t.append(bt)
        b1s = const.tile([128, MC], F32, name="b1s", tag="b1s")
        nc.sync.dma_start(out=b1s, in_=b1s_io[:, :])

        x_sb = [persist.tile([128, H], F32, name=f"x{sb}", tag=f"x{sb}")
                for sb in range(SB)]
        x2_sb = [persist.tile([128, H], F32, name=f"x2_{sb}", tag=f"x2_{sb}")
                 for sb in range(SB)]
        qtall = persist.tile([HD, NH, SL], FP8, name="qtall", tag="qtall")
        attn_nat = [persist.tile([128, H], BF16, name=f"an{sb}",
                                 tag=f"an{sb}") for sb in range(SB)]
        attnT8 = persist.tile([128, HC, SL], FP8, name="attnT8",
                              tag="attnT8")

        # ============ phase A: LN1, QKV (fp8 DoubleRow), RoPE ============
        p_ln = tc.alloc_tile_pool(name="p_ln", bufs=1)
        p_qkv = tc.alloc_tile_pool(name="p_qkv", bufs=1)
        wqkv = tc.alloc_tile_pool(name="wqkv", bufs=1)
        psA_tr = tc.alloc_tile_pool(name="psA_tr", bufs=3, space="PSUM")
        psA_mm = tc.alloc_tile_pool(name="psA_mm", bufs=2, space="PSUM")

        for sb in range(SB):
            nc.scalar.dma_start(out=x_sb[sb],
                                in_=x_io[sb * 128:(sb + 1) * 128, :])
        cosr = [p_qkv.tile([128, H], BF16, name=f"cos{sb}", tag=f"cos{sb}")
                for sb in range(SB)]
        sins = [p_qkv.tile([128, H], BF16, name=f"sin{sb}", tag=f"sin{sb}")
                for sb in range(SB)]
        wk8 = wqkv.tile([128, HC, H], FP8, name="wk8", tag="wk8")
        wv8 = wqkv.tile([128, HC, H], FP8, name="wv8", tag="wv8")
        wq8 = wqkv.tile([128, HC, H], FP8, name="wq8", tag="wq8")
        nc.sync.dma_start(out=wk8,
                          in_=wk_io.rearrange("(t p) f -> p t f", p=128))
        nc.gpsimd.dma_start(out=wv8,
                            in_=wv_io.rearrange("(t p) f -> p t f", p=128))
        nc.gpsimd.dma_start(out=wq8,
                            in_=wq_io.rearrange("(t p) f -> p t f", p=128))
        for sb in range(SB):
            nc.sync.dma_start(out=cosr[sb],
                              in_=cos_io[sb * 128:(sb + 1) * 128, :])
            nc.sync.dma_start(out=sins[sb],
                              in_=sin_io[sb * 128:(sb + 1) * 128, :])

        xln = [p_ln.tile([128, H], BF16, name=f"xln{sb}", tag=f"xln{sb}")
               for sb in range(SB)]
        for sb in range(SB):
            stats = misc.tile([128, 5, 6], F32, name=f"lnst{sb}", tag="lnst")
            sv = x_sb[sb].rearrange("p (g d) -> p g d", d=256)
            for g in range(5):
                nc.vector.bn_stats(out=stats[:, g, :], in_=sv[:, g, :])
            mv = misc.tile([128, 2], F32, name=f"lnmv{sb}", tag="lnmv")
            nc.vector.bn_aggr(out=mv, in_=stats)
            rstd = misc.tile([128, 1], F32, name=f"lnrs{sb}", tag="lnrs")
            nc.scalar.activation(out=rstd, in_=mv[:, 1:2], func=AF.Sqrt,
                                 bias=eps_t)
            nc.vector.reciprocal(out=rstd, in_=rstd)
            nc.vector.tensor_scalar(out=xln[sb], in0=x_sb[sb],
                                    scalar1=mv[:, 0:1], scalar2=rstd,
                                    op0=OP.subtract, op1=OP.mult)

        # packed stationary LN1 output, fp8 [128, HC, SL]
        xlnT8 = p_ln.tile([128, HC, SL], FP8, name="xlnT8", tag="xlnT8")
        for hc in range(HC):
            pt = psA_tr.tile([128, 256], BF16, name="pt", tag="pt")
            for sb in range(SB):
                nc.tensor.transpose(pt[:, sb * 128:(sb + 1) * 128],
                                    xln[sb][:, hc * 128:(hc + 1) * 128],
                                    ident)
            nc.scalar.copy(out=xlnT8[:, hc, :], in_=pt)

        def project_dr(w8, bias_idx, cols, emit):
            # emit(sb, c0, cn, ps) consumes the accumulated PSUM chunk
            for sb in range(SB):
                for (c0, cn) in cols:
                    ps = psA_mm.tile([128, 512], F32, name="mmps", tag="mmps")
                    for t in range(DR):
                        nc.tensor.matmul(
                            ps[:, 0:cn],
                            lhsT=xlnT8[:, 2 * t:2 * t + 2,
                                       sb * 128:(sb + 1) * 128],
                            rhs=w8[:, 2 * t:2 * t + 2, c0:c0 + cn],
                            start=(t == 0),
                            stop=(not use_bias and t == DR - 1),
                            perf_mode=DRPM)
                    if use_bias:
                        nc.tensor.matmul(
                            ps[:, 0:cn], lhsT=ones_b,
                            rhs=bias_t[bias_idx][:, c0:c0 + cn],
                            start=False, stop=True)
                    emit(sb, c0, cn, ps)

        def rope(nat, out):
            for sb in range(SB):
                tmp = p_qkv.tile([128, H], BF16, name=f"ropetmp{sb}",
                                 tag=f"ropetmp{sb}")
                t3 = tmp.rearrange("p (h c) -> p h c", c=HD)
                q3 = nat[sb].rearrange("p (h c) -> p h c", c=HD)
                s3 = sins[sb].rearrange("p (h c) -> p h c", c=HD)
                nc.vector.tensor_mul(out=t3[:, :, 0:40], in0=q3[:, :, 40:80],
                                     in1=s3[:, :, 0:40])
                nc.vector.tensor_mul(out=t3[:, :, 40:80], in0=q3[:, :, 0:40],
                                     in1=s3[:, :, 40:80])
                nc.vector.tensor_mul(out=nat[sb], in0=nat[sb], in1=cosr[sb])
                nc.vector.tensor_add(out=out[sb], in0=nat[sb], in1=tmp)

        def head_transpose(src, dst):
            # src: SB tiles [128, H] bf16 -> dst [HD, NH, SL] fp8
            for h in range(NH):
                ptk = psA_tr.tile([HD, 256], BF16, name="ptk", tag="ptk")
                for sb in range(SB):
                    nc.tensor.transpose(ptk[:, sb * 128:(sb + 1) * 128],
                                        src[sb][:, h * HD:(h + 1) * HD],
                                        ident)
                nc.vector.tensor_copy(out=dst[:, h, :], in_=ptk)

        # ---- K first: project, rope, transpose, publish ----
        knat = [p_qkv.tile([128, H], BF16, name=f"kn{sb}", tag=f"kn{sb}")
                for sb in range(SB)]
        project_dr(wk8, 1, NCOLS,
                   lambda sb, c0, cn, ps: nc.scalar.activation(
                       out=knat[sb][:, c0:c0 + cn], in_=ps[:, 0:cn],
                       func=AF.Copy, scale=1.0 / WS))
        krope = [p_qkv.tile([128, H], BF16, name=f"kr{sb}", tag=f"kr{sb}")
                 for sb in range(SB)]
        rope(knat, krope)
        ktloc = p_qkv.tile([HD, NH, SL], FP8, name="ktloc", tag="ktloc")
        head_transpose(krope, ktloc)
        pub_k = nc.sync.dma_start(
            out=cc_in[0:KT_ELEMS].rearrange("(d h s) -> d h s", d=HD, h=NH),
            in_=ktloc)

        # ---- V: project straight to fp8 with denominator column ----
        vloc = p_qkv.tile([128, SB, NH, HD1], FP8, name="vloc", tag="vloc")
        nc.vector.memset(vloc[:, :, :, HD:HD1], 1.0 / AS)

        def emit_v(sb, c0, cn, ps):
            h0 = c0 // HD
            nh = cn // HD
            nc.scalar.activation(
                out=vloc[:, sb, h0:h0 + nh, 0:HD],
                in_=ps[:, 0:cn].rearrange("p (h c) -> p h c", c=HD),
                func=AF.Copy, scale=1.0 / WS)

        project_dr(wv8, 2, VCOLS, emit_v)
        pub_v = nc.scalar.dma_start(
            out=cc_in[KT_ELEMS:KVE].rearrange("(lb p x) -> p lb x",
                                              lb=SB, p=128),
            in_=vloc.rearrange("p lb h c -> p lb (h c)"))
        bar_kv = nc.gpsimd.collective_compute(
            "AllGather", OP.bypass,
            replica_groups=[list(range(NCORES))],
            ins=[cc_in.ap()], outs=[cc_out.ap()])
        bass_rust.add_dep_helper(bar_kv.ins, pub_k.ins,
                                 reason="gather after K publish")
        bass_rust.add_dep_helper(bar_kv.ins, pub_v.ins,
                                 reason="gather after V publish")

        # ---- Q: project, rope, transpose (overlaps the AllGather) ----
        qnat = [p_qkv.tile([128, H], BF16, name=f"qn{sb}", tag=f"qn{sb}")
                for sb in range(SB)]
        project_dr(wq8, 0, NCOLS,
                   lambda sb, c0, cn, ps: nc.scalar.activation(
                       out=qnat[sb][:, c0:c0 + cn], in_=ps[:, 0:cn],
                       func=AF.Copy, scale=1.0 / WS))
        qrope = [p_qkv.tile([128, H], BF16, name=f"qr{sb}", tag=f"qr{sb}")
                 for sb in range(SB)]
        rope(qnat, qrope)
        head_transpose(qrope, qtall)

        psA_mm.release()
        psA_tr.release()
        wqkv.release()
        p_qkv.release()
        p_ln.release()

        # ============ phase B: attention ============
        p_att = tc.alloc_tile_pool(name="p_att", bufs=1)
        katt = tc.alloc_tile_pool(name="katt", bufs=3)
        eatt = tc.alloc_tile_pool(name="eatt", bufs=4)
        w1p = tc.alloc_tile_pool(name="w1p", bufs=1, side="right")
        wop = tc.alloc_tile_pool(name="wop", bufs=1, side="right")
        ps_sc = tc.alloc_tile_pool(name="ps_sc", bufs=1, space="PSUM")
        ps_at = tc.alloc_tile_pool(name="ps_at", bufs=2, space="PSUM")
        ps_tr2 = tc.alloc_tile_pool(name="ps_tr2", bufs=2, space="PSUM")

        v_all = p_att.tile([128, KB, NH * HD1], FP8, name="v_all",
                           tag="v_all")
        w1g = [w1p.tile([128, 5, HC, 128], BF16, name=f"w1g{g}",
                        tag=f"w1g{g}") for g in range(W1G)]
        wo8 = wop.tile([128, HC, H], FP8, name="wo8t", tag="wo8t")

        # MLP/O weight prefetch rides the scalar queue during the gather
        for g in range(W1G):
            nc.scalar.dma_start(out=w1g[g], in_=w1b_io[
                g * 5:(g + 1) * 5].rearrange("j p t m -> p j t m"))
        nc.scalar.dma_start(out=wo8, in_=wo_io.rearrange(
            "(t p) f -> p t f", p=128))

        # remote V lands on the gpsimd queue once AG-V completes
        vg = cc_out[:, KT_ELEMS:KVE].rearrange("r (lb p x) -> p r lb x",
                                               lb=SB, p=128)
        for r in range(NCORES):
            vdma = nc.gpsimd.dma_start(
                out=v_all[:, r * SB:(r + 1) * SB, :], in_=vg[:, r, :, :])
            bass_rust.add_dep_helper(vdma.ins, bar_kv.ins,
                                     reason="remote V after gather")

        kall = cc_out[:, 0:KT_ELEMS].rearrange("r (d h s) -> d h r s",
                                               d=HD, h=NH)
        e_tiles = {}
        LAG = 2
        # last head whose normalize completes column block hc of attn_nat
        HC_LAST = [(((hc + 1) * 128 + HD - 1) // HD) - 1 for hc in range(HC)]

        def attn_v(h):
            e_h = e_tiles.pop(h)
            pa = [ps_at.tile([128, 96], F32, name=f"atps{qb}", tag="atps")
                  for qb in range(SB)]
            for kb in range(KB):
                for qb in range(SB):
                    nc.tensor.matmul(
                        pa[qb][:, 0:HD1],
                        lhsT=e_h[:, kb, qb * 128:(qb + 1) * 128],
                        rhs=v_all[:, kb, h * HD1:(h + 1) * HD1],
                        start=(kb == 0), stop=(kb == KB - 1))
            for qb in range(SB):
                zrec = misc.tile([128, 1], F32, name="zrec", tag="zrec")
                nc.vector.reciprocal(out=zrec, in_=pa[qb][:, HD:HD1])
                nc.vector.tensor_scalar_mul(
                    attn_nat[qb][:, h * HD:(h + 1) * HD],
                    pa[qb][:, 0:HD], zrec)
            # evacuate finished attn_nat column blocks into the fp8
            # stationary for the O projection while attention continues
            for hc in range(HC):
                if HC_LAST[hc] != h:
                    continue
                pt = ps_tr2.tile([128, 256], BF16, name="pti", tag="pti")
                for sb in range(SB):
                    nc.tensor.transpose(
                        pt[:, sb * 128:(sb + 1) * 128],
                        attn_nat[sb][:, hc * 128:(hc + 1) * 128], ident)
                if hc % 2 == 0:
                    nc.vector.tensor_copy(out=attnT8[:, hc, :], in_=pt)
                else:
                    nc.scalar.copy(out=attnT8[:, hc, :], in_=pt)

        for h in range(NH):
            eng = nc.sync if h < 10 else nc.gpsimd
            kt = katt.tile([HD, NCORES * SL], FP8, name=f"kt{h}", tag="kt")
            kdma = eng.dma_start(out=kt.rearrange("d (r s) -> d r s", s=SL),
                                 in_=kall[:, h, :, :])
            bass_rust.add_dep_helper(kdma.ins, bar_kv.ins,
                                     reason="remote K after gather")

            e_h = eatt.tile([128, KB, SL], FP8, name=f"e{h}", tag="eh")
            e_tiles[h] = e_h
            for k8 in range(KB // 8):
                ps = ps_sc.tile([128, 8 * SL], F32, name="scps", tag="scps")
                for j in range(8):
                    kb = k8 * 8 + j
                    nc.tensor.matmul(ps[:, j * SL:(j + 1) * SL],
                                     lhsT=kt[:, kb * 128:(kb + 1) * 128],
                                     rhs=qtall[:, h, :],
                                     start=True, stop=True)
                ev = e_h[:, k8 * 8:(k8 + 1) * 8, :]
                nc.scalar.activation(
                    out=ev, in_=ps.rearrange("p (a b) -> p a b", b=SL),
                    func=AF.Exp, scale=SCALE)
                if k8 == 0 and h >= LAG:
                    attn_v(h - LAG)
        for h in range(NH - LAG, NH):
            attn_v(h)

        ps_tr2.release()
        ps_at.release()
        ps_sc.release()
        eatt.release()
        katt.release()
        p_att.release()

        # ============ phase C: O projection (fp8 DR) + LN2 ============
        p_c = tc.alloc_tile_pool(name="p_c", bufs=1)
        w2p = tc.alloc_tile_pool(name="w2p", bufs=3)
        psC_tr = tc.alloc_tile_pool(name="psC_tr", bufs=2, space="PSUM")
        psC_mm = tc.alloc_tile_pool(name="psC_mm", bufs=2, space="PSUM")

        w2tiles = {}

        def w2_dma(g, eng):
            w2 = w2p.tile([128, 4, H], BF16, name=f"w2g{g}", tag="w2g")
            eng.dma_start(
                out=w2,
                in_=w2t_io[g * 512:(g + 1) * 512, :].rearrange(
                    "(k p) f -> p k f", p=128))
            w2tiles[g] = w2

        w2_dma(0, nc.sync)
        w2_dma(1, nc.gpsimd)

        for sb in range(SB):
            for (c0, cn) in NCOLS:
                ps = psC_mm.tile([128, 512], F32, name="ops", tag="ops")
                for t in range(DR):
                    nc.tensor.matmul(
                        ps[:, 0:cn],
                        lhsT=attnT8[:, 2 * t:2 * t + 2,
                                    sb * 128:(sb + 1) * 128],
                        rhs=wo8[:, 2 * t:2 * t + 2, c0:c0 + cn],
                        start=(t == 0),
                        stop=(not use_bias and t == DR - 1),
                        perf_mode=DRPM)
                if use_bias:
                    nc.tensor.matmul(ps[:, 0:cn], lhsT=ones_b,
                                     rhs=bias_t[3][:, c0:c0 + cn],
                                     start=False, stop=True)
                ot = p_c.tile([128, 512], F32, name="otmp", tag="otmp")
                nc.scalar.activation(out=ot[:, 0:cn], in_=ps[:, 0:cn],
                                     func=AF.Copy, scale=1.0 / (WS * AS))
                nc.vector.tensor_add(out=x2_sb[sb][:, c0:c0 + cn],
                                     in0=ot[:, 0:cn],
                                     in1=x_sb[sb][:, c0:c0 + cn])

        xln2 = [p_c.tile([128, H], BF16, name=f"xln2{sb}", tag=f"xln2{sb}")
                for sb in range(SB)]
        for sb in range(SB):
            stats = misc.tile([128, 5, 6], F32, name=f"ln2st{sb}", tag="lnst")
            sv = x2_sb[sb].rearrange("p (g d) -> p g d", d=256)
            for g in range(5):
                nc.vector.bn_stats(out=stats[:, g, :], in_=sv[:, g, :])
            mv = misc.tile([128, 2], F32, name=f"ln2mv{sb}", tag="lnmv")
            nc.vector.bn_aggr(out=mv, in_=stats)
            rstd = misc.tile([128, 1], F32, name=f"ln2rs{sb}", tag="lnrs")
            nc.scalar.activation(out=rstd, in_=mv[:, 1:2], func=AF.Sqrt,
                                 bias=eps_t)
            nc.vector.reciprocal(out=rstd, in_=rstd)
            nc.vector.tensor_scalar(out=xln2[sb], in0=x2_sb[sb],
                                    scalar1=mv[:, 0:1], scalar2=rstd,
                                    op0=OP.subtract, op1=OP.mult)

        xln2T = p_c.tile([128, HC, SL], BF16, name="xln2T", tag="xln2T")
        for hc in range(HC):
            pt = psC_tr.tile([128, 256], BF16, name="ptc2", tag="ptc")
            for sb in range(SB):
                nc.tensor.transpose(pt[:, sb * 128:(sb + 1) * 128],
                                    xln2[sb][:, hc * 128:(hc + 1) * 128],
                                    ident)
            if hc % 2 == 0:
                nc.vector.tensor_copy(out=xln2T[:, hc, :], in_=pt)
            else:
                nc.scalar.copy(out=xln2T[:, hc, :], in_=pt)

        psC_mm.release()
        psC_tr.release()

        # ============ phase D: MLP (bf16) ============
        gtp = tc.alloc_tile_pool(name="gtp", bufs=3)
        ps_fc1 = tc.alloc_tile_pool(name="ps_fc1", bufs=2, space="PSUM")
        ps_fc2 = tc.alloc_tile_pool(name="ps_fc2", bufs=1, space="PSUM")

        fc2ps = {}
        for sb in range(SB):
            for (c0, cn) in NCOLS:
                fc2ps[(sb, c0)] = ps_fc2.tile([128, 512], F32,
                                              name=f"fc2ps{sb}_{c0}",
                                              tag=f"fc2ps{sb}_{c0}")
        for mb in range(MC):
            g, j = divmod(mb, 5)
            wg, wj = divmod(mb, 4)
            if wj == 0 and 2 + wg < W2G:
                w2_dma(2 + wg, nc.sync if wg % 2 == 0 else nc.gpsimd)
            p1 = ps_fc1.tile([128, SL], F32, name="fc1ps", tag="fc1ps")
            for hc in range(HC):
                nc.tensor.matmul(p1, lhsT=w1g[g][:, j, hc, :],
                                 rhs=xln2T[:, hc, :],
                                 start=(hc == 0), stop=(hc == HC - 1))
            gt = gtp.tile([128, SL], BF16, name=f"gt{mb}", tag="gt")
            nc.scalar.activation(out=gt, in_=p1, func=AF.Silu,
                                 scale=1.702, bias=b1s[:, mb:mb + 1])
            for sb in range(SB):
                for (c0, cn) in NCOLS:
                    nc.tensor.matmul(fc2ps[(sb, c0)][:, 0:cn],
                                     lhsT=gt[:, sb * 128:(sb + 1) * 128],
                                     rhs=w2tiles[wg][:, wj, c0:c0 + cn],
                                     start=(mb == 0),
                                     stop=(not use_bias and mb == MC - 1))
        for sb in range(SB):
            for (c0, cn) in NCOLS:
                if use_bias:
                    nc.tensor.matmul(fc2ps[(sb, c0)][:, 0:cn], lhsT=ones_b,
                                     rhs=bias_t[4][:, c0:c0 + cn],
                                     start=False, stop=True)
                nc.vector.tensor_add(out=x_sb[sb][:, c0:c0 + cn],
                                     in0=fc2ps[(sb, c0)][:, 0:cn],
                                     in1=x2_sb[sb][:, c0:c0 + cn])
            nc.sync.dma_start(out=out_io[sb * 128:(sb + 1) * 128, :],
                              in_=x_sb[sb])

        ps_fc2.release()
        ps_fc1.release()
        gtp.release()
        w2p.release()
        wop.release()
        w1p.release()
        p_c.release()
        misc.release()
        persist.release()
        const.release()

    nc.compile()
    return nc


_NC = {}


def _get_nc(use_bias=False):
    if use_bias not in _NC:
        _NC[use_bias] = _build_bass(use_bias)
    return _NC[use_bias]


def _prep_inputs(hidden_states, cos, sin,
                 ln1_g, ln1_b, ln2_g, ln2_b,
                 Wq, bq, Wk, bk, Wv, bv, Wo, bo,
                 W1, b1, W2, b2):
    f32 = np.float32
    x = np.asarray(hidden_states, f32).reshape(S, H)
    cos = np.asarray(cos, f32)
    sin = np.asarray(sin, f32)
    g1 = np.asarray(ln1_g, f32); be1 = np.asarray(ln1_b, f32)
    g2 = np.asarray(ln2_g, f32); be2 = np.asarray(ln2_b, f32)
    Wq = np.asarray(Wq, f32); Wk = np.asarray(Wk, f32); Wv = np.asarray(Wv, f32)
    Wo = np.asarray(Wo, f32); W1 = np.asarray(W1, f32); W2 = np.asarray(W2, f32)

    # fold LN1 affine into QKV, LN2 affine into fc1 (exact in fp32);
    # scale fp8 weights by WS so they sit in e4m3's normal range
    wq8 = (g1[:, None] * Wq.T * WS).astype(F8)
    wk8 = (g1[:, None] * Wk.T * WS).astype(F8)
    wv8 = (g1[:, None] * Wv.T * WS).astype(F8)
    wo8 = (Wo.T * WS).astype(F8)
    bq_e = np.asarray(bq, f32) + Wq @ be1
    bk_e = np.asarray(bk, f32) + Wk @ be1
    bv_e = np.asarray(bv, f32) + Wv @ be1
    w1t = g2[:, None] * W1.T                       # [H, MLP]
    w1b = np.ascontiguousarray(
        w1t.reshape(HC, 128, MC, 128).transpose(2, 1, 0, 3)).astype(BF)
    b1_e = np.asarray(b1, f32) + W1 @ be2
    b1s = np.ascontiguousarray(
        (1.702 * b1_e).reshape(MC, 128).T).astype(f32)  # [128, MC]
    w2t = (W2.T / 1.702).astype(BF)                 # gelu scale folded
    bias5 = np.stack([bq_e * WS, bk_e * WS, bv_e * WS,
                      np.asarray(bo, f32) * WS * AS,
                      np.asarray(b2, f32)]).astype(BF)

    cos_rep = np.tile(cos, (1, NH)).astype(BF)      # [S, H]
    sin_sgn = np.concatenate([-sin[:, :40], sin[:, 40:]], axis=1)
    sin_rep = np.tile(sin_sgn, (1, NH)).astype(BF)  # [S, H]

    shared = {
        "wq8": wq8, "wk8": wk8, "wv8": wv8, "wo8": wo8,
        "w1b": w1b, "w2t": w2t, "bias5": bias5, "b1s": b1s,
    }
    in_maps = []
    for c in range(NCORES):
        sl = slice(c * SL, (c + 1) * SL)
        m = dict(shared)
        m["x_loc"] = np.ascontiguousarray(x[sl])
        m["cosr"] = np.ascontiguousarray(cos_rep[sl])
        m["sins"] = np.ascontiguousarray(sin_rep[sl])
        in_maps.append(m)
    return in_maps


def kernel(hidden_states, attention_mask, cos, sin,
           ln1_g, ln1_b, ln2_g, ln2_b,
           Wq, bq, Wk, bk, Wv, bv, Wo, bo,
           W1, b1, W2, b2):
    # attention_mask is all-True for this problem (spec fill: ones); the
    # dense softmax below assumes it.
    from concourse.bass_utils import run_bass_kernel_spmd

    use_bias = any(
        float(np.abs(np.asarray(b, np.float32)).max()) != 0.0
        for b in (bq, bk, bv, bo, b2))
    nc = _get_nc(use_bias)
    in_maps = _prep_inputs(hidden_states, cos, sin,
                           ln1_g, ln1_b, ln2_g, ln2_b,
                           Wq, bq, Wk, bk, Wv, bv, Wo, bo,
                           W1, b1, W2, b2)
    res = run_bass_kernel_spmd(nc, in_maps, core_ids=list(range(NCORES)))
    out = np.concatenate([res.results[c]["out_loc"] for c in range(NCORES)],
                         axis=0)
    return out.reshape(B, S, H).astype(np.float32)


# revision 25
# speedup vs baseline: 1.0972x; 1.0972x over previous
"""Trainium2 Bass kernel for a Qwen2-VL vision transformer block.

Strategy: 8-way sequence-parallel across NeuronCores. Each core owns a
256-row shard of the 2048-token sequence and the full weights. K/V for
the full sequence are exchanged with a single fp8 AllGather; every other
stage is perfectly partitioned.

Precision plan (rel-err budget 2e-2, measured ~2e-3):
  - Residual stream and LayerNorm statistics in fp32.
  - MLP matmuls in bf16 (dominant error term, ~1.7e-3).
  - QKV and O projections in fp8e4m3 with DoubleRow perf mode (2x PE
    throughput, half weight DMA). Weights are pre-scaled by 64 on the
    host so they sit in e4m3's normal range; the 1/64 is folded into the
    PSUM->SBUF copies.
  - Attention (Q/K/V/exp(scores)) in fp8e4m3: halves the AllGather and
    the K/V reload traffic. Error contribution is ~3e-4 at the output
    because attn_out is small relative to the residual.
  - The softmax denominator is accumulated via an extra (1/32)-valued
    column appended to each head's V in the gathered layout; attn is
    scaled by 32 into bf16 (normal range for the later fp8 cast) and the
    32*64 is folded into the O-projection PSUM evacuation.

Layout notes:
  - All projections use a packed stationary activation tile
    [128, HC, seq] so DoubleRow can slice two 128-row contraction tiles
    per instruction; weights load as single [128, HC, H] DMAs.
  - Attention computes scores^T [key, query] per head (exp on the
    scalar engine straight out of PSUM), then attnV with exp(scores)
    stationary, producing attention in natural [query, head*HD] layout.
  - W1 is fully prefetched into SBUF during attention (DMA is otherwise
    idle there); W2 streams during the MLP with grouped DMAs.
"""

import sys

import numpy as np

for _p in ("/opt/trn_rl_repo",):
    if _p not in sys.path:
        sys.path.insert(0, _p)

import ml_dtypes  # noqa: E402


BF = ml_dtypes.bfloat16
F8 = ml_dtypes.float8_e4m3

B, S, H = 1, 2048, 1280
NH, HD = 16, 80
MLP = 5120
EPS = 1e-6
NCORES = 8
SL = S // NCORES            # 256 sequence rows per core
SB = SL // 128              # 2 partition blocks per core
HC = H // 128               # 10 contraction chunks over H
DR = HC // 2                # 5 DoubleRow steps over H
MC = MLP // 128             # 40 blocks over the MLP dim
KB = S // 128               # 16 key blocks over the full sequence
NCOLS = ((0, 512), (512, 512), (1024, 256))
VCOLS = ((0, 480), (480, 480), (960, 320))   # head-aligned chunks for V
SCALE = 1.0 / float(np.sqrt(np.float32(HD)))
WS = 64.0                   # host-side fp8 weight scale for QKV/O
AS = 32.0                   # on-chip attention scale (normalize trick)
HD1 = HD + 1                # V head stride incl. denominator column
KT_ELEMS = NH * HD * SL     # 327680, gathered K^T region (fp8 bytes)
V_ELEMS = SL * NH * HD1     # 331776, gathered V(+ones) region
KVE = KT_ELEMS + V_ELEMS
W1G = 8                     # W1 prefetch groups (5 mb-tiles each)
W2G = 10                    # W2 stream groups (4 mb-tiles each)


def _build_bass(use_bias):
    import bass_rust
    import concourse.bacc as bacc
    import concourse.tile as tile
    from concourse import mybir
    from concourse.masks import make_identity

    F32 = mybir.dt.float32
    BF16 = mybir.dt.bfloat16
    FP8 = mybir.dt.float8e4
    AF = mybir.ActivationFunctionType
    OP = mybir.AluOpType
    DRPM = mybir.MatmulPerfMode.DoubleRow

    nc = bacc.Bacc("TRN2", target_bir_lowering=False, debug=False,
                   num_devices=NCORES)

    x_io = nc.dram_tensor("x_loc", [SL, H], F32, kind="ExternalInput")
    cos_io = nc.dram_tensor("cosr", [SL, H], BF16, kind="ExternalInput")
    sin_io = nc.dram_tensor("sins", [SL, H], BF16, kind="ExternalInput")
    wq_io = nc.dram_tensor("wq8", [H, H], FP8, kind="ExternalInput")
    wk_io = nc.dram_tensor("wk8", [H, H], FP8, kind="ExternalInput")
    wv_io = nc.dram_tensor("wv8", [H, H], FP8, kind="ExternalInput")
    wo_io = nc.dram_tensor("wo8", [H, H], FP8, kind="ExternalInput")
    w1b_io = nc.dram_tensor("w1b", [MC, 128, HC, 128], BF16,
                            kind="ExternalInput")
    w2t_io = nc.dram_tensor("w2t", [MLP, H], BF16, kind="ExternalInput")
    bias5_io = nc.dram_tensor("bias5", [5, H], BF16, kind="ExternalInput")
    b1sn_io = nc.dram_tensor("b1sn", [128, MC], F32, kind="ExternalInput")
    b1n_io = nc.dram_tensor("b1n", [128, MC], F32, kind="ExternalInput")
    out_io = nc.dram_tensor("out_loc", [SL, H], F32, kind="ExternalOutput")

    cc_in = nc.dram_tensor("cc_in", [KVE], FP8)
    cc_out = nc.dram_tensor("cc_out", [NCORES, KVE], FP8,
                            addr_space="Shared")

    with tile.TileContext(nc) as tc:
        const = tc.alloc_tile_pool(name="const", bufs=1)
        persist = tc.alloc_tile_pool(name="persist", bufs=1)
        misc = tc.alloc_tile_pool(name="misc", bufs=2)

        ident = const.tile([128, 128], BF16, name="ident", tag="ident")
        make_identity(nc, ident)
        ones_b = const.tile([1, 128], BF16, name="ones_b", tag="ones_b")
        nc.vector.memset(ones_b, 1.0)
        eps_t = const.tile([128, 1], F32, name="eps_t", tag="eps_t")
        nc.vector.memset(eps_t, EPS)
        bias_t = []
        if use_bias:
            for bi in range(5):
                bt = const.tile([1, H], BF16, name=f"bias{bi}",
                                tag=f"bias{bi}")
                nc.sync.dma_start(out=bt, in_=bias5_io[bi:bi + 1, :])
                bias_t.append(bt)
        b1sn = const.tile([128, MC], F32, name="b1sn", tag="b1sn")
        nc.sync.dma_start(out=b1sn, in_=b1sn_io[:, :])
        b1n = const.tile([128, MC], F32, name="b1n", tag="b1n")
        nc.sync.dma_start(out=b1n, in_=b1n_io[:, :])
        one_t = const.tile([128, 1], F32, name="one_t", tag="one_t")
        nc.vector.memset(one_t, 1.0)

        x_sb = [persist.tile([128, H], F32, name=f"x{sb}", tag=f"x{sb}")
                for sb in range(SB)]
        x2_sb = [persist.tile([128, H], F32, name=f"x2_{sb}", tag=f"x2_{sb}")
                 for sb in range(SB)]
        qtall = persist.tile([HD, NH, SL], FP8, name="qtall", tag="qtall")
        attn_nat = [persist.tile([128, H], BF16, name=f"an{sb}",
                                 tag=f"an{sb}") for sb in range(SB)]
        attnT8 = persist.tile([128, HC, SL], FP8, name="attnT8",
                              tag="attnT8")

        # ============ phase A: LN1, QKV (fp8 DoubleRow), RoPE ============
        p_ln = tc.alloc_tile_pool(name="p_ln", bufs=1)
        p_qkv = tc.alloc_tile_pool(name="p_qkv", bufs=1)
        wqkv = tc.alloc_tile_pool(name="wqkv", bufs=1)
        psA_tr = tc.alloc_tile_pool(name="psA_tr", bufs=3, space="PSUM")
        psA_mm = tc.alloc_tile_pool(name="psA_mm", bufs=2, space="PSUM")

        for sb in range(SB):
            nc.scalar.dma_start(out=x_sb[sb],
                                in_=x_io[sb * 128:(sb + 1) * 128, :])
        cosr = [p_qkv.tile([128, H], BF16, name=f"cos{sb}", tag=f"cos{sb}")
                for sb in range(SB)]
        sins = [p_qkv.tile([128, H], BF16, name=f"sin{sb}", tag=f"sin{sb}")
                for sb in range(SB)]
        wk8 = wqkv.tile([128, HC, H], FP8, name="wk8", tag="wk8")
        wv8 = wqkv.tile([128, HC, H], FP8, name="wv8", tag="wv8")
        wq8 = wqkv.tile([128, HC, H], FP8, name="wq8", tag="wq8")
        nc.sync.dma_start(out=wk8,
                          in_=wk_io.rearrange("(t p) f -> p t f", p=128))
        nc.gpsimd.dma_start(out=wv8,
                            in_=wv_io.rearrange("(t p) f -> p t f", p=128))
        nc.gpsimd.dma_start(out=wq8,
                            in_=wq_io.rearrange("(t p) f -> p t f", p=128))
        for sb in range(SB):
            nc.sync.dma_start(out=cosr[sb],
                              in_=cos_io[sb * 128:(sb + 1) * 128, :])
            nc.sync.dma_start(out=sins[sb],
                              in_=sin_io[sb * 128:(sb + 1) * 128, :])

        xln = [p_ln.tile([128, H], BF16, name=f"xln{sb}", tag=f"xln{sb}")
               for sb in range(SB)]
        for sb in range(SB):
            stats = misc.tile([128, 5, 6], F32, name=f"lnst{sb}", tag="lnst")
            sv = x_sb[sb].rearrange("p (g d) -> p g d", d=256)
            for g in range(5):
                nc.vector.bn_stats(out=stats[:, g, :], in_=sv[:, g, :])
            mv = misc.tile([128, 2], F32, name=f"lnmv{sb}", tag="lnmv")
            nc.vector.bn_aggr(out=mv, in_=stats)
            rstd = misc.tile([128, 1], F32, name=f"lnrs{sb}", tag="lnrs")
            nc.scalar.activation(out=rstd, in_=mv[:, 1:2], func=AF.Sqrt,
                                 bias=eps_t)
            nc.vector.reciprocal(out=rstd, in_=rstd)
            nc.vector.tensor_scalar(out=xln[sb], in0=x_sb[sb],
                                    scalar1=mv[:, 0:1], scalar2=rstd,
                                    op0=OP.subtract, op1=OP.mult)

        # packed stationary LN1 output, fp8 [128, HC, SL]
        xlnT8 = p_ln.tile([128, HC, SL], FP8, name="xlnT8", tag="xlnT8")
        for hc in range(HC):
            pt = psA_tr.tile([128, 256], BF16, name="pt", tag="pt")
            for sb in range(SB):
                nc.tensor.transpose(pt[:, sb * 128:(sb + 1) * 128],
                                    xln[sb][:, hc * 128:(hc + 1) * 128],
                                    ident)
            nc.scalar.copy(out=xlnT8[:, hc, :], in_=pt)

        def project_dr(w8, bias_idx, cols, emit):
            # emit(sb, c0, cn, ps) consumes the accumulated PSUM chunk
            for sb in range(SB):
                for (c0, cn) in cols:
                    ps = psA_mm.tile([128, 512], F32, name="mmps", tag="mmps")
                    for t in range(DR):
                        nc.tensor.matmul(
                            ps[:, 0:cn],
                            lhsT=xlnT8[:, 2 * t:2 * t + 2,
                                       sb * 128:(sb + 1) * 128],
                            rhs=w8[:, 2 * t:2 * t + 2, c0:c0 + cn],
                            start=(t == 0),
                            stop=(not use_bias and t == DR - 1),
                            perf_mode=DRPM)
                    if use_bias:
                        nc.tensor.matmul(
                            ps[:, 0:cn], lhsT=ones_b,
                            rhs=bias_t[bias_idx][:, c0:c0 + cn],
                            start=False, stop=True)
                    emit(sb, c0, cn, ps)

        def rope(nat, out):
            for sb in range(SB):
                tmp = p_qkv.tile([128, H], BF16, name=f"ropetmp{sb}",
                                 tag=f"ropetmp{sb}")
                t3 = tmp.rearrange("p (h c) -> p h c", c=HD)
                q3 = nat[sb].rearrange("p (h c) -> p h c", c=HD)
                s3 = sins[sb].rearrange("p (h c) -> p h c", c=HD)
                nc.vector.tensor_mul(out=t3[:, :, 0:40], in0=q3[:, :, 40:80],
                                     in1=s3[:, :, 0:40])
                nc.vector.tensor_mul(out=t3[:, :, 40:80], in0=q3[:, :, 0:40],
                                     in1=s3[:, :, 40:80])
                nc.vector.tensor_mul(out=nat[sb], in0=nat[sb], in1=cosr[sb])
                nc.vector.tensor_add(out=out[sb], in0=nat[sb], in1=tmp)

        def head_transpose(src, dst):
            # src: SB tiles [128, H] bf16 -> dst [HD, NH, SL] fp8
            for h in range(NH):
                ptk = psA_tr.tile([HD, 256], BF16, name="ptk", tag="ptk")
                for sb in range(SB):
                    nc.tensor.transpose(ptk[:, sb * 128:(sb + 1) * 128],
                                        src[sb][:, h * HD:(h + 1) * HD],
                                        ident)
                nc.vector.tensor_copy(out=dst[:, h, :], in_=ptk)

        # ---- K first: project, rope, transpose, publish ----
        knat = [p_qkv.tile([128, H], BF16, name=f"kn{sb}", tag=f"kn{sb}")
                for sb in range(SB)]
        project_dr(wk8, 1, NCOLS,
                   lambda sb, c0, cn, ps: nc.scalar.activation(
                       out=knat[sb][:, c0:c0 + cn], in_=ps[:, 0:cn],
                       func=AF.Copy, scale=1.0 / WS))
        krope = [p_qkv.tile([128, H], BF16, name=f"kr{sb}", tag=f"kr{sb}")
                 for sb in range(SB)]
        rope(knat, krope)
        ktloc = p_qkv.tile([HD, NH, SL], FP8, name="ktloc", tag="ktloc")
        head_transpose(krope, ktloc)
        pub_k = nc.sync.dma_start(
            out=cc_in[0:KT_ELEMS].rearrange("(d h s) -> d h s", d=HD, h=NH),
            in_=ktloc)

        # ---- V: project straight to fp8 with denominator column ----
        vloc = p_qkv.tile([128, SB, NH, HD1], FP8, name="vloc", tag="vloc")
        nc.vector.memset(vloc[:, :, :, HD:HD1], 1.0 / AS)

        def emit_v(sb, c0, cn, ps):
            h0 = c0 // HD
            nh = cn // HD
            nc.scalar.activation(
                out=vloc[:, sb, h0:h0 + nh, 0:HD],
                in_=ps[:, 0:cn].rearrange("p (h c) -> p h c", c=HD),
                func=AF.Copy, scale=1.0 / WS)

        project_dr(wv8, 2, VCOLS, emit_v)
        pub_v = nc.scalar.dma_start(
            out=cc_in[KT_ELEMS:KVE].rearrange("(lb p x) -> p lb x",
                                              lb=SB, p=128),
            in_=vloc.rearrange("p lb h c -> p lb (h c)"))
        bar_kv = nc.gpsimd.collective_compute(
            "AllGather", OP.bypass,
            replica_groups=[list(range(NCORES))],
            ins=[cc_in.ap()], outs=[cc_out.ap()])
        bass_rust.add_dep_helper(bar_kv.ins, pub_k.ins,
                                 reason="gather after K publish")
        bass_rust.add_dep_helper(bar_kv.ins, pub_v.ins,
                                 reason="gather after V publish")

        # ---- Q: project, rope, transpose (overlaps the AllGather) ----
        qnat = [p_qkv.tile([128, H], BF16, name=f"qn{sb}", tag=f"qn{sb}")
                for sb in range(SB)]
        project_dr(wq8, 0, NCOLS,
                   lambda sb, c0, cn, ps: nc.scalar.activation(
                       out=qnat[sb][:, c0:c0 + cn], in_=ps[:, 0:cn],
                       func=AF.Copy, scale=1.0 / WS))
        qrope = [p_qkv.tile([128, H], BF16, name=f"qr{sb}", tag=f"qr{sb}")
                 for sb in range(SB)]
        rope(qnat, qrope)
        head_transpose(qrope, qtall)

        psA_mm.release()
        psA_tr.release()
        wqkv.release()
        p_qkv.release()
        p_ln.release()

        # ============ phase B: attention ============
        p_att = tc.alloc_tile_pool(name="p_att", bufs=1)
        katt = tc.alloc_tile_pool(name="katt", bufs=3)
        eatt = tc.alloc_tile_pool(name="eatt", bufs=4)
        w1p = tc.alloc_tile_pool(name="w1p", bufs=1, side="right")
        wop = tc.alloc_tile_pool(name="wop", bufs=1, side="right")
        ps_sc = tc.alloc_tile_pool(name="ps_sc", bufs=2, space="PSUM")
        ps_at = tc.alloc_tile_pool(name="ps_at", bufs=2, space="PSUM")
        ps_tr2 = tc.alloc_tile_pool(name="ps_tr2", bufs=2, space="PSUM")

        v_all = p_att.tile([128, KB, NH * HD1], FP8, name="v_all",
                           tag="v_all")
        w1g = [w1p.tile([128, 5, HC, 128], BF16, name=f"w1g{g}",
                        tag=f"w1g{g}") for g in range(W1G)]
        wo8 = wop.tile([128, HC, H], FP8, name="wo8t", tag="wo8t")

        # MLP/O weight prefetch rides the scalar queue during the gather
        for g in range(W1G):
            nc.scalar.dma_start(out=w1g[g], in_=w1b_io[
                g * 5:(g + 1) * 5].rearrange("j p t m -> p j t m"))
        nc.scalar.dma_start(out=wo8, in_=wo_io.rearrange(
            "(t p) f -> p t f", p=128))

        # remote V lands on the gpsimd queue once AG-V completes
        vg = cc_out[:, KT_ELEMS:KVE].rearrange("r (lb p x) -> p r lb x",
                                               lb=SB, p=128)
        for r in range(NCORES):
            vdma = nc.gpsimd.dma_start(
                out=v_all[:, r * SB:(r + 1) * SB, :], in_=vg[:, r, :, :])
            bass_rust.add_dep_helper(vdma.ins, bar_kv.ins,
                                     reason="remote V after gather")

        kall = cc_out[:, 0:KT_ELEMS].rearrange("r (d h s) -> d h r s",
                                               d=HD, h=NH)
        e_tiles = {}
        LAG = 2
        # last head whose normalize completes column block hc of attn_nat
        HC_LAST = [(((hc + 1) * 128 + HD - 1) // HD) - 1 for hc in range(HC)]

        def attn_v(h):
            e_h = e_tiles.pop(h)
            pa = [ps_at.tile([128, 96], F32, name=f"atps{qb}", tag="atps")
                  for qb in range(SB)]
            for kb in range(KB):
                for qb in range(SB):
                    nc.tensor.matmul(
                        pa[qb][:, 0:HD1],
                        lhsT=e_h[:, kb, qb * 128:(qb + 1) * 128],
                        rhs=v_all[:, kb, h * HD1:(h + 1) * HD1],
                        start=(kb == 0), stop=(kb == KB - 1))
            for qb in range(SB):
                zrec = misc.tile([128, 1], F32, name="zrec", tag="zrec")
                nc.vector.reciprocal(out=zrec, in_=pa[qb][:, HD:HD1])
                nc.vector.tensor_scalar_mul(
                    attn_nat[qb][:, h * HD:(h + 1) * HD],
                    pa[qb][:, 0:HD], zrec)
            # evacuate finished attn_nat column blocks into the fp8
            # stationary for the O projection while attention continues
            for hc in range(HC):
                if HC_LAST[hc] != h:
                    continue
                pt = ps_tr2.tile([128, 256], BF16, name="pti", tag="pti")
                for sb in range(SB):
                    nc.tensor.transpose(
                        pt[:, sb * 128:(sb + 1) * 128],
                        attn_nat[sb][:, hc * 128:(hc + 1) * 128], ident)
                if hc % 2 == 0:
                    nc.vector.tensor_copy(out=attnT8[:, hc, :], in_=pt)
                else:
                    nc.scalar.copy(out=attnT8[:, hc, :], in_=pt)

        for h in range(NH):
            eng = nc.sync if h < 10 else nc.gpsimd
            kt = katt.tile([HD, NCORES * SL], FP8, name=f"kt{h}", tag="kt")
            kdma = eng.dma_start(out=kt.rearrange("d (r s) -> d r s", s=SL),
                                 in_=kall[:, h, :, :])
            bass_rust.add_dep_helper(kdma.ins, bar_kv.ins,
                                     reason="remote K after gather")

            e_h = eatt.tile([128, KB, SL], FP8, name=f"e{h}", tag="eh")
            e_tiles[h] = e_h
            for k4 in range(KB // 4):
                ps = ps_sc.tile([128, 4 * SL], F32, name="scps", tag="scps")
                for j in range(4):
                    kb = k4 * 4 + j
                    nc.tensor.matmul(ps[:, j * SL:(j + 1) * SL],
                                     lhsT=kt[:, kb * 128:(kb + 1) * 128],
                                     rhs=qtall[:, h, :],
                                     start=True, stop=True)
                ev = e_h[:, k4 * 4:(k4 + 1) * 4, :]
                nc.scalar.activation(
                    out=ev, in_=ps.rearrange("p (a b) -> p a b", b=SL),
                    func=AF.Exp, scale=SCALE)
                if k4 == 1 and h >= LAG:
                    attn_v(h - LAG)
        for h in range(NH - LAG, NH):
            attn_v(h)

        ps_tr2.release()
        ps_at.release()
        ps_sc.release()
        eatt.release()
        katt.release()
        p_att.release()

        # ============ phase C: O projection (fp8 DR) + LN2 ============
        p_c = tc.alloc_tile_pool(name="p_c", bufs=1)
        w2p = tc.alloc_tile_pool(name="w2p", bufs=3)
        psC_tr = tc.alloc_tile_pool(name="psC_tr", bufs=2, space="PSUM")
        psC_mm = tc.alloc_tile_pool(name="psC_mm", bufs=2, space="PSUM")

        w2tiles = {}

        def w2_dma(g, eng):
            w2 = w2p.tile([128, 4, H], BF16, name=f"w2g{g}", tag="w2g")
            eng.dma_start(
                out=w2,
                in_=w2t_io[g * 512:(g + 1) * 512, :].rearrange(
                    "(k p) f -> p k f", p=128))
            w2tiles[g] = w2

        w2_dma(0, nc.sync)
        w2_dma(1, nc.gpsimd)

        for sb in range(SB):
            for (c0, cn) in NCOLS:
                ps = psC_mm.tile([128, 512], F32, name="ops", tag="ops")
                for t in range(DR):
                    nc.tensor.matmul(
                        ps[:, 0:cn],
                        lhsT=attnT8[:, 2 * t:2 * t + 2,
                                    sb * 128:(sb + 1) * 128],
                        rhs=wo8[:, 2 * t:2 * t + 2, c0:c0 + cn],
                        start=(t == 0),
                        stop=(not use_bias and t == DR - 1),
                        perf_mode=DRPM)
                if use_bias:
                    nc.tensor.matmul(ps[:, 0:cn], lhsT=ones_b,
                                     rhs=bias_t[3][:, c0:c0 + cn],
                                     start=False, stop=True)
                ot = p_c.tile([128, 512], F32, name="otmp", tag="otmp")
                nc.scalar.activation(out=ot[:, 0:cn], in_=ps[:, 0:cn],
                                     func=AF.Copy, scale=1.0 / (WS * AS))
                nc.vector.tensor_add(out=x2_sb[sb][:, c0:c0 + cn],
                                     in0=ot[:, 0:cn],
                                     in1=x_sb[sb][:, c0:c0 + cn])

        xln2 = [p_c.tile([128, H], BF16, name=f"xln2{sb}", tag=f"xln2{sb}")
                for sb in range(SB)]
        for sb in range(SB):
            stats = misc.tile([128, 5, 6], F32, name=f"ln2st{sb}", tag="lnst")
            sv = x2_sb[sb].rearrange("p (g d) -> p g d", d=256)
            for g in range(5):
                nc.vector.bn_stats(out=stats[:, g, :], in_=sv[:, g, :])
            mv = misc.tile([128, 2], F32, name=f"ln2mv{sb}", tag="lnmv")
            nc.vector.bn_aggr(out=mv, in_=stats)
            rstd = misc.tile([128, 1], F32, name=f"ln2rs{sb}", tag="lnrs")
            nc.scalar.activation(out=rstd, in_=mv[:, 1:2], func=AF.Sqrt,
                                 bias=eps_t)
            nc.vector.reciprocal(out=rstd, in_=rstd)
            nc.vector.tensor_scalar(out=xln2[sb], in0=x2_sb[sb],
                                    scalar1=mv[:, 0:1], scalar2=rstd,
                                    op0=OP.subtract, op1=OP.mult)

        xln2T = p_c.tile([128, HC, SL], BF16, name="xln2T", tag="xln2T")
        for hc in range(HC):
            pt = psC_tr.tile([128, 256], BF16, name="ptc2", tag="ptc")
            for sb in range(SB):
                nc.tensor.transpose(pt[:, sb * 128:(sb + 1) * 128],
                                    xln2[sb][:, hc * 128:(hc + 1) * 128],
                                    ident)
            if hc % 2 == 0:
                nc.vector.tensor_copy(out=xln2T[:, hc, :], in_=pt)
            else:
                nc.scalar.copy(out=xln2T[:, hc, :], in_=pt)

        psC_mm.release()
        psC_tr.release()

        # ============ phase D: MLP (bf16) ============
        gtp = tc.alloc_tile_pool(name="gtp", bufs=3)
        ps_fc1 = tc.alloc_tile_pool(name="ps_fc1", bufs=2, space="PSUM")
        ps_fc2 = tc.alloc_tile_pool(name="ps_fc2", bufs=1, space="PSUM")

        fc2ps = {}
        for sb in range(SB):
            for (c0, cn) in NCOLS:
                fc2ps[(sb, c0)] = ps_fc2.tile([128, 512], F32,
                                              name=f"fc2ps{sb}_{c0}",
                                              tag=f"fc2ps{sb}_{c0}")
        gts = {}
        for mb in range(MC + 1):
            if mb < MC:
                g, j = divmod(mb, 5)
                wg, wj = divmod(mb, 4)
                if wj == 0 and 2 + wg < W2G:
                    w2_dma(2 + wg, nc.sync if wg % 2 == 0 else nc.gpsimd)
                p1 = ps_fc1.tile([128, SL], F32, name="fc1ps", tag="fc1ps")
                for hc in range(HC):
                    nc.tensor.matmul(p1, lhsT=w1g[g][:, j, hc, :],
                                     rhs=xln2T[:, hc, :],
                                     start=(hc == 0), stop=(hc == HC - 1))
                # quickgelu(z) = z * sigmoid(1.702 z), z = p1 + b1.  exp on
                # the scalar engine is ~1 cyc/elem where Silu's table is ~8;
                # the elementwise tail rides the otherwise-idle vector engine
                et = gtp.tile([128, SL], BF16, name="et", tag="et")
                nc.scalar.activation(out=et, in_=p1, func=AF.Exp,
                                     scale=-1.702, bias=b1sn[:, mb:mb + 1])
                dt = gtp.tile([128, SL], F32, name="dt", tag="dt")
                nc.vector.tensor_scalar(out=dt, in0=et, scalar1=one_t,
                                        scalar2=None, op0=OP.add)
                rt = gtp.tile([128, SL], F32, name="rt", tag="rt")
                nc.vector.reciprocal(out=rt, in_=dt)
                zt = gtp.tile([128, SL], F32, name="zt", tag="zt")
                nc.vector.tensor_scalar(out=zt, in0=p1,
                                        scalar1=b1n[:, mb:mb + 1],
                                        scalar2=None, op0=OP.add)
                gt = gtp.tile([128, SL], BF16, name=f"gt{mb}", tag="gt")
                nc.vector.tensor_mul(out=gt, in0=zt, in1=rt)
                gts[mb] = gt
            if mb >= 1:
                pm = mb - 1
                pwg, pwj = divmod(pm, 4)
                gt = gts.pop(pm)
                for sb in range(SB):
                    for (c0, cn) in NCOLS:
                        nc.tensor.matmul(
                            fc2ps[(sb, c0)][:, 0:cn],
                            lhsT=gt[:, sb * 128:(sb + 1) * 128],
                            rhs=w2tiles[pwg][:, pwj, c0:c0 + cn],
                            start=(pm == 0),
                            stop=(not use_bias and pm == MC - 1))
        for sb in range(SB):
            for (c0, cn) in NCOLS:
                if use_bias:
                    nc.tensor.matmul(fc2ps[(sb, c0)][:, 0:cn], lhsT=ones_b,
                                     rhs=bias_t[4][:, c0:c0 + cn],
                                     start=False, stop=True)
                nc.vector.tensor_add(out=x_sb[sb][:, c0:c0 + cn],
                                     in0=fc2ps[(sb, c0)][:, 0:cn],
                                     in1=x2_sb[sb][:, c0:c0 + cn])
            nc.sync.dma_start(out=out_io[sb * 128:(sb + 1) * 128, :],
                              in_=x_sb[sb])

        ps_fc2.release()
        ps_fc1.release()
        gtp.release()
        w2p.release()
        wop.release()
        w1p.release()
        p_c.release()
        misc.release()
        persist.release()
        const.release()

    nc.compile()
    return nc


_NC = {}


def _get_nc(use_bias=False):
    if use_bias not in _NC:
        _NC[use_bias] = _build_bass(use_bias)
    return _NC[use_bias]


def _prep_inputs(hidden_states, cos, sin,
                 ln1_g, ln1_b, ln2_g, ln2_b,
                 Wq, bq, Wk, bk, Wv, bv, Wo, bo,
                 W1, b1, W2, b2):
    f32 = np.float32
    x = np.asarray(hidden_states, f32).reshape(S, H)
    cos = np.asarray(cos, f32)
    sin = np.asarray(sin, f32)
    g1 = np.asarray(ln1_g, f32); be1 = np.asarray(ln1_b, f32)
    g2 = np.asarray(ln2_g, f32); be2 = np.asarray(ln2_b, f32)
    Wq = np.asarray(Wq, f32); Wk = np.asarray(Wk, f32); Wv = np.asarray(Wv, f32)
    Wo = np.asarray(Wo, f32); W1 = np.asarray(W1, f32); W2 = np.asarray(W2, f32)

    # fold LN1 affine into QKV, LN2 affine into fc1 (exact in fp32);
    # scale fp8 weights by WS so they sit in e4m3's normal range
    wq8 = (g1[:, None] * Wq.T * WS).astype(F8)
    wk8 = (g1[:, None] * Wk.T * WS).astype(F8)
    wv8 = (g1[:, None] * Wv.T * WS).astype(F8)
    wo8 = (Wo.T * WS).astype(F8)
    bq_e = np.asarray(bq, f32) + Wq @ be1
    bk_e = np.asarray(bk, f32) + Wk @ be1
    bv_e = np.asarray(bv, f32) + Wv @ be1
    w1t = g2[:, None] * W1.T                       # [H, MLP]
    w1b = np.ascontiguousarray(
        w1t.reshape(HC, 128, MC, 128).transpose(2, 1, 0, 3)).astype(BF)
    b1_e = np.asarray(b1, f32) + W1 @ be2
    b1sn = np.ascontiguousarray(
        (-1.702 * b1_e).reshape(MC, 128).T).astype(f32)  # [128, MC]
    b1n = np.ascontiguousarray(
        b1_e.reshape(MC, 128).T).astype(f32)             # [128, MC]
    w2t = W2.T.astype(BF)
    bias5 = np.stack([bq_e * WS, bk_e * WS, bv_e * WS,
                      np.asarray(bo, f32) * WS * AS,
                      np.asarray(b2, f32)]).astype(BF)

    cos_rep = np.tile(cos, (1, NH)).astype(BF)      # [S, H]
    sin_sgn = np.concatenate([-sin[:, :40], sin[:, 40:]], axis=1)
    sin_rep = np.tile(sin_sgn, (1, NH)).astype(BF)  # [S, H]

    shared = {
        "wq8": wq8, "wk8": wk8, "wv8": wv8, "wo8": wo8,
        "w1b": w1b, "w2t": w2t, "bias5": bias5,
        "b1sn": b1sn, "b1n": b1n,
    }
    in_maps = []
    for c in range(NCORES):
        sl = slice(c * SL, (c + 1) * SL)
        m = dict(shared)
        m["x_loc"] = np.ascontiguousarray(x[sl])
        m["cosr"] = np.ascontiguousarray(cos_rep[sl])
        m["sins"] = np.ascontiguousarray(sin_rep[sl])
        in_maps.append(m)
    return in_maps


def kernel(hidden_states, attention_mask, cos, sin,
           ln1_g, ln1_b, ln2_g, ln2_b,
           Wq, bq, Wk, bk, Wv, bv, Wo, bo,
           W1, b1, W2, b2):
    # attention_mask is all-True for this problem (spec fill: ones); the
    # dense softmax below assumes it.
    from concourse.bass_utils import run_bass_kernel_spmd

    use_bias = any(
        float(np.abs(np.asarray(b, np.float32)).max()) != 0.0
        for b in (bq, bk, bv, bo, b2))
    nc = _get_nc(use_bias)
    in_maps = _prep_inputs(hidden_states, cos, sin,
                           ln1_g, ln1_b, ln2_g, ln2_b,
                           Wq, bq, Wk, bk, Wv, bv, Wo, bo,
                           W1, b1, W2, b2)
    res = run_bass_kernel_spmd(nc, in_maps, core_ids=list(range(NCORES)))
    out = np.concatenate([res.results[c]["out_loc"] for c in range(NCORES)],
                         axis=0)
    return out.reshape(B, S, H).astype(np.float32)
